# revision 26
# baseline (speedup 1.0000x reference)
"""Trainium2 Bass kernel for nn_Discriminator2 (bilinear discriminator scores).

Math: with hc0 = h_c[0] [N, D], W0 = W[0] [D, D]:
    v      = hc0 @ W0.T                      [N, D]   (tensor engine, bf16)
    sc1[n] = dot(h_pl[0][n], v[n]) + b       [N]      (fused DVE mult+reduce)
    sc2[s,n] = dot(hc0[sample[s,n]], v[n]) + b        (indirect-DMA gather + DVE/ACT)
    out    = [sc1 | sc2.flat | sc2.flat]     [1, N + 2*S*N]

Sharding: nodes (N) split evenly across 8 cores; hc0 replicated on every core
so gathers resolve locally; W replicated; h_pl / sample_list sharded by node.

Engine budget per 128-node tile (measured on HW):
  Pool (SWDGE desc-gen): 4 indirect gathers x ~1.40us effective (1.09us
    ucode busy + ~0.31us fixed instruction-boundary gap) <- THE critical
    path: 392 calls/core = ~550us. HW honors ONE index per partition per
    call (descriptor p moves out-row-length contiguous bytes from row
    idx[p]), so 4 calls/tile is irreducible. All batched-gather
    instructions (InstDMAGatherAnt, InstTensorTensorReduce) are
    Anthropic-extended ucode ABSENT from this runtime - they hard-crash
    the exec unit (NRT_EXEC_UNIT_UNRECOVERABLE). Tested 2026-08-11.
  DVE: 2 fused scalar_tensor_tensor dots (~0.8us ea, accum_out) +
    3 multiplies vs PSUM v (~0.7us ea).
  ACT: 3 Copy-activation accum reductions.
  PE: 4 accumulating bf16 matmuls (v = hc_tile @ W.T).
"""

import sys

for _p in ("/opt/trn_rl_repo",):
    if _p not in sys.path:
        sys.path.insert(0, _p)

import ml_dtypes
import numpy as np

import concourse.bass as bass
import concourse.mybir as mybir
import concourse.tile as tile
from concourse import bacc
from concourse.bass_utils import run_bass_kernel_spmd

P = 128  # partitions


class Cfg:
    """Problem geometry. Full-size defaults; shrink for CoreSim validation."""

    def __init__(self, n_table=100000, nodes_per_core=12500, d=512, s=4,
                 n_cores=8, super_tile=4, n_queues=1):
        self.n_table = n_table          # rows of the gather table (full N)
        self.nodes_per_core = nodes_per_core
        self.d = d
        self.s = s
        self.n_cores = n_cores
        self.super_tile = super_tile    # node-tiles per hcT DMA block
        self.n_queues = n_queues
        self.tiles = -(-nodes_per_core // P)        # ceil
        self.npad = self.tiles * P
        self.kc = d // P                # contraction chunks


FULL = Cfg()


def build_nc(cfg: Cfg):
    D, S, KC, TILES = cfg.d, cfg.s, cfg.kc, cfg.tiles
    bf16 = mybir.dt.bfloat16
    f32 = mybir.dt.float32

    # 64KB descriptor-ring carveout (default 16KB): one 128-row indirect
    # gather fills the default ring, serializing descriptor prep against
    # drain; a deeper ring lets several calls queue per SWDGE queue.
    nc = bacc.Bacc("TRN2", target_bir_lowering=False, debug=False,
                   num_swdge_queues=cfg.n_queues,
                   dynamic_dma_scratch_size=131072)
    # bf16 table: the SWDGE indirect gather moves bf16 rows, halving the
    # gather HBM stream vs f32 (102.8 -> 51.4 MB/core).
    hc = nc.dram_tensor("hc", [cfg.n_table, D], bf16,
                        kind="ExternalInput").ap()
    hcT = nc.dram_tensor("hcT", [D, cfg.npad], bf16, kind="ExternalInput").ap()
    hpl = nc.dram_tensor("hpl", [cfg.npad, D], bf16,
                         kind="ExternalInput").ap()
    idx = nc.dram_tensor("idx", [P, TILES * S], mybir.dt.int32,
                         kind="ExternalInput").ap()
    wt = nc.dram_tensor("wt", [D, D], bf16, kind="ExternalInput").ap()
    out = nc.dram_tensor("out", [P, TILES * (S + 1)], f32,
                         kind="ExternalOutput").ap()

    with tile.TileContext(nc) as tc:
        with (
            tc.tile_pool(name="const", bufs=1) as cpool,
            tc.tile_pool(name="hcT", bufs=2) as hcT_pool,
            tc.tile_pool(name="hpl", bufs=8) as hpl_pool,
            tc.tile_pool(name="g", bufs=14) as g_pool,
            tc.tile_pool(name="prod", bufs=8) as prod_pool,
            tc.tile_pool(name="psum", bufs=6, space="PSUM") as psum_pool,
        ):
            # All gather indices resident: idx_sb[p, t*S+s] = sample[s, t*128+p].
            # Loaded FIRST so the gather stream (the kernel's critical path)
            # starts as early as possible.
            idx_sb = cpool.tile([P, TILES * S], mybir.dt.int32)
            # head-split: the first super-tile's gathers wait only on a tiny
            # 8KB DMA instead of the full 200KB index load
            head = cfg.super_tile * S
            nc.sync.dma_start(out=idx_sb[:, :head], in_=idx[:, :head])
            nc.sync.dma_start(out=idx_sb[:, head:], in_=idx[:, head:])
            # W.T resident: free layout (c, d) — chunk c covers contraction
            # rows c*128..c*128+127.
            wt_sb = cpool.tile([P, KC * D], bf16)
            nc.sync.dma_start(
                out=wt_sb[:].rearrange("p (c d) -> p c d", c=KC),
                in_=wt.rearrange("(c p) d -> p c d", p=P))
            sc_acc = cpool.tile([P, TILES * (S + 1)], f32)
            dump = cpool.tile([P, D], bf16)  # discarded ACT elementwise output

            for t0 in range(0, TILES, cfg.super_tile):
                st = min(cfg.super_tile, TILES - t0)
                # hcT block [D, st*128] -> SBUF free layout (c, n_local)
                hcT_sb = hcT_pool.tile([P, KC * cfg.super_tile * P], bf16,
                                       tag="hcT")
                nc.sync.dma_start(
                    out=hcT_sb[:, : KC * st * P].rearrange(
                        "p (c n) -> p c n", c=KC),
                    in_=hcT[:, t0 * P:(t0 + st) * P].rearrange(
                        "(c p) n -> p c n", p=P),
                )
                for j in range(st):
                    t = t0 + j
                    hpl_sb = hpl_pool.tile([P, D], bf16, tag="hpl")
                    nc.sync.dma_start(out=hpl_sb[:],
                                      in_=hpl[t * P:(t + 1) * P, :])
                    # Gather the S sampled rows per node (HW indirect DMA
                    # honors one index per partition, so one call per s):
                    # g_sb[p, s*D:(s+1)*D] = hc[idx_sb[p, t*S+s], :]
                    g_sb = g_pool.tile([P, S * D], bf16, tag="g")
                    for s in range(S):
                        gi = nc.gpsimd.indirect_dma_start(
                            out=g_sb[:, s * D:(s + 1) * D],
                            out_offset=None,
                            in_=hc[:],
                            in_offset=bass.IndirectOffsetOnAxis(
                                ap=idx_sb[:, t * S + s:t * S + s + 1], axis=0),
                        )
                        # spread calls across SWDGE queues so SDMA
                        # interleaves descriptor streams
                        q = (t * S + s) % cfg.n_queues
                        if q:
                            gi.ins.queue = f"qPoolDynamic{q}"
                    # v = hc0_tile @ W.T via 4 accumulating bf16 matmuls
                    v_ps = psum_pool.tile([P, D], f32, space="PSUM", tag="v_ps")
                    for c in range(KC):
                        off = (c * st + j) * P
                        nc.tensor.matmul(
                            out=v_ps[:],
                            lhsT=hcT_sb[:, off:off + P],
                            rhs=wt_sb[:, c * D:(c + 1) * D],
                            start=(c == 0),
                            stop=(c == KC - 1),
                        )
                    col = t * (S + 1)
                    # sc1 and g0: fused multiply+reduce on DVE (one DVE op,
                    # no ACT involvement; InstTensorScalarPtr w/ accum_out)
                    for s, in0 in ((0, hpl_sb[:]), (1, g_sb[:, 0:D])):
                        stt_out = prod_pool.tile([P, D], bf16, tag="prod")
                        nc.vector.scalar_tensor_tensor(
                            out=stt_out[:],
                            in0=in0,
                            scalar=1.0,
                            in1=v_ps[:],
                            op0=mybir.AluOpType.mult,
                            op1=mybir.AluOpType.mult,
                            accum_out=sc_acc[:, col + s:col + s + 1],
                        )
                    # g1..g3: bf16 2x-mode multiply on DVE, reduce on ACT via
                    # Copy-activation accum_out
                    for s in range(2, S + 1):
                        prod = prod_pool.tile([P, D], bf16, tag="prod")
                        nc.vector.tensor_mul(prod[:], g_sb[:, (s - 1) * D:s * D],
                                             v_ps[:])
                        nc.scalar.activation(
                            dump[:], prod[:],
                            mybir.ActivationFunctionType.Copy,
                            accum_out=sc_acc[:, col + s:col + s + 1],
                        )
            nc.sync.dma_start(out=out[:], in_=sc_acc[:])
    nc.compile()
    return nc


def make_in_maps(cfg: Cfg, h_c, h_pl, sample_list, W, b):
    """Host-side sharding: full inputs -> per-core input dicts."""
    D, S = cfg.d, cfg.s
    hc0 = np.ascontiguousarray(np.asarray(h_c, np.float32)[0])
    hpl0 = np.asarray(h_pl, np.float32)[0]
    smp = np.asarray(sample_list)
    W0 = np.asarray(W, np.float32)[0]
    bval = float(np.asarray(b, np.float32).reshape(-1)[0])

    hc_bf = hc0.astype(ml_dtypes.bfloat16)             # gather table
    hcT = np.ascontiguousarray(hc0.T).astype(ml_dtypes.bfloat16)   # [D, N]
    wt = np.ascontiguousarray(W0.T).astype(ml_dtypes.bfloat16)

    in_maps = []
    for c in range(cfg.n_cores):
        lo = c * cfg.nodes_per_core
        hi = lo + cfg.nodes_per_core
        hcT_s = np.zeros((D, cfg.npad), ml_dtypes.bfloat16)
        hcT_s[:, : cfg.nodes_per_core] = hcT[:, lo:hi]
        hpl_s = np.zeros((cfg.npad, D), ml_dtypes.bfloat16)
        hpl_s[: cfg.nodes_per_core] = hpl0[lo:hi].astype(ml_dtypes.bfloat16)
        idx_s = np.zeros((S, cfg.npad), np.int64)
        idx_s[:, : cfg.nodes_per_core] = smp[:, lo:hi]
        idx_r = np.ascontiguousarray(
            idx_s.reshape(S, cfg.tiles, P).transpose(2, 1, 0)
            .astype(np.int32).reshape(P, cfg.tiles * S))
        in_maps.append({
            "hc": hc_bf, "hcT": hcT_s, "hpl": hpl_s,
            "idx": idx_r, "wt": wt,
        })
    return in_maps, bval


def assemble_output(cfg: Cfg, outs, bval):
    """Per-core 'out' arrays [P, TILES*(S+1)] -> full logits [1, N + 2*S*N].

    The +b bias is applied here (host-side) instead of on-device."""
    S = cfg.s
    n = cfg.nodes_per_core * cfg.n_cores
    sc1 = np.empty((n,), np.float32)
    sc2 = np.empty((S, n), np.float32)
    for c in range(cfg.n_cores):
        o = (outs[c].reshape(P, cfg.tiles, S + 1).transpose(2, 1, 0)
             .reshape(S + 1, cfg.npad)[:, : cfg.nodes_per_core])
        lo = c * cfg.nodes_per_core
        sc1[lo:lo + cfg.nodes_per_core] = o[0]
        sc2[:, lo:lo + cfg.nodes_per_core] = o[1:]
    flat = sc2.reshape(-1)
    res = np.concatenate([sc1, flat, flat])[None, :]
    if bval != 0.0:
        res = res + bval
    return res.astype(np.float32)


_NC_CACHE = {}


def _get_nc(cfg: Cfg):
    key = (cfg.n_table, cfg.nodes_per_core, cfg.d, cfg.s, cfg.super_tile,
           cfg.n_queues)
    if key not in _NC_CACHE:
        _NC_CACHE[key] = build_nc(cfg)
    return _NC_CACHE[key]


def run_on_hw(cfg: Cfg, inputs, trace=False, trace_kwargs={}):
    nc = _get_nc(cfg)
    in_maps, bval = make_in_maps(cfg, **inputs)
    res = run_bass_kernel_spmd(nc, in_maps, core_ids=list(range(cfg.n_cores)),
                               trace=trace, trace_kwargs=trace_kwargs)
    out = assemble_output(cfg, [r["out"] for r in res.results], bval)
    return out, res


def kernel(h_c, h_pl, sample_list, W, b):
    inputs = dict(h_c=h_c, h_pl=h_pl, sample_list=sample_list, W=W, b=b)
    out, _ = run_on_hw(FULL, inputs, trace=False)
    return out


if __name__ == "__main__":
    import reference

    inputs = reference.setup_inputs()
    expected = np.asarray(reference.reference(**inputs))
    got = kernel(**{k: np.asarray(v) for k, v in inputs.items()})
    rel = np.abs(got - expected).max() / np.abs(expected).max()
    print("Relative error:", rel)


# revision 27
# speedup vs baseline: 1.0076x; 1.0076x over previous
"""Trainium2 Bass kernel for nn_Discriminator2 (bilinear discriminator scores).

Math: with hc0 = h_c[0] [N, D], W0 = W[0] [D, D]:
    v      = hc0 @ W0.T                      [N, D]   (tensor engine, bf16)
    sc1[n] = dot(h_pl[0][n], v[n]) + b       [N]      (fused DVE mult+reduce)
    sc2[s,n] = dot(hc0[sample[s,n]], v[n]) + b        (indirect-DMA gather + DVE/ACT)
    out    = [sc1 | sc2.flat | sc2.flat]     [1, N + 2*S*N]

Sharding: nodes (N) split evenly across 8 cores; hc0 replicated on every core
so gathers resolve locally; W replicated; h_pl / sample_list sharded by node.

Engine budget per 128-node tile (measured on HW):
  Pool (SWDGE desc-gen): 4 indirect gathers x ~1.40us effective (1.09us
    ucode busy + ~0.31us fixed instruction-boundary gap) <- THE critical
    path: 392 calls/core = ~550us. HW honors ONE index per partition per
    call (descriptor p moves out-row-length contiguous bytes from row
    idx[p]), so 4 calls/tile is irreducible. All batched-gather
    instructions (InstDMAGatherAnt, InstTensorTensorReduce) are
    Anthropic-extended ucode ABSENT from this runtime - they hard-crash
    the exec unit (NRT_EXEC_UNIT_UNRECOVERABLE). Tested 2026-08-11.
  DVE: 2 fused scalar_tensor_tensor dots (~0.8us ea, accum_out) +
    3 multiplies vs PSUM v (~0.7us ea).
  ACT: 3 Copy-activation accum reductions.
  PE: 4 accumulating bf16 matmuls (v = hc_tile @ W.T).
"""

import sys

for _p in ("/opt/trn_rl_repo",):
    if _p not in sys.path:
        sys.path.insert(0, _p)

import ml_dtypes
import numpy as np

import concourse.bass as bass
import concourse.mybir as mybir
import concourse.tile as tile
from concourse import bacc
from concourse.bass_utils import run_bass_kernel_spmd

P = 128  # partitions


class Cfg:
    """Problem geometry. Full-size defaults; shrink for CoreSim validation."""

    def __init__(self, n_table=100000, nodes_per_core=12500, d=512, s=4,
                 n_cores=8, super_tile=4, n_queues=1):
        self.n_table = n_table          # rows of the gather table (full N)
        self.nodes_per_core = nodes_per_core
        self.d = d
        self.s = s
        self.n_cores = n_cores
        self.super_tile = super_tile    # node-tiles per hcT DMA block
        self.n_queues = n_queues
        self.tiles = -(-nodes_per_core // P)        # ceil
        self.npad = self.tiles * P
        self.kc = d // P                # contraction chunks


FULL = Cfg()


def build_nc(cfg: Cfg):
    D, S, KC, TILES = cfg.d, cfg.s, cfg.kc, cfg.tiles
    bf16 = mybir.dt.bfloat16
    f32 = mybir.dt.float32

    # 64KB descriptor-ring carveout (default 16KB): one 128-row indirect
    # gather fills the default ring, serializing descriptor prep against
    # drain; a deeper ring lets several calls queue per SWDGE queue.
    nc = bacc.Bacc("TRN2", target_bir_lowering=False, debug=False,
                   num_swdge_queues=cfg.n_queues,
                   dynamic_dma_scratch_size=131072)
    # bf16 table: the SWDGE indirect gather moves bf16 rows, halving the
    # gather HBM stream vs f32 (102.8 -> 51.4 MB/core).
    hc = nc.dram_tensor("hc", [cfg.n_table, D], bf16,
                        kind="ExternalInput").ap()
    hcT = nc.dram_tensor("hcT", [D, cfg.npad], bf16, kind="ExternalInput").ap()
    hpl = nc.dram_tensor("hpl", [cfg.npad, D], bf16,
                         kind="ExternalInput").ap()
    idx = nc.dram_tensor("idx", [P, TILES * S], mybir.dt.int32,
                         kind="ExternalInput").ap()
    wt = nc.dram_tensor("wt", [D, D], bf16, kind="ExternalInput").ap()
    out = nc.dram_tensor("out", [P, TILES * (S + 1)], f32,
                         kind="ExternalOutput").ap()

    with tile.TileContext(nc) as tc:
        with (
            tc.tile_pool(name="const", bufs=1) as cpool,
            tc.tile_pool(name="hcT", bufs=2) as hcT_pool,
            tc.tile_pool(name="hpl", bufs=8) as hpl_pool,
            tc.tile_pool(name="g", bufs=14) as g_pool,
            tc.tile_pool(name="prod", bufs=8) as prod_pool,
            tc.tile_pool(name="psum", bufs=6, space="PSUM") as psum_pool,
        ):
            # All gather indices resident: idx_sb[p, t*S+s] = sample[s, t*128+p].
            # Loaded FIRST so the gather stream (the kernel's critical path)
            # starts as early as possible.
            idx_sb = cpool.tile([P, TILES * S], mybir.dt.int32)
            nc.sync.dma_start(out=idx_sb[:], in_=idx[:])
            # W.T resident: free layout (c, d) — chunk c covers contraction
            # rows c*128..c*128+127.
            wt_sb = cpool.tile([P, KC * D], bf16)
            nc.sync.dma_start(
                out=wt_sb[:].rearrange("p (c d) -> p c d", c=KC),
                in_=wt.rearrange("(c p) d -> p c d", p=P))
            sc_acc = cpool.tile([P, TILES * (S + 1)], f32)
            dump = cpool.tile([P, D], bf16)  # discarded ACT elementwise output

            for t0 in range(0, TILES, cfg.super_tile):
                st = min(cfg.super_tile, TILES - t0)
                # hcT block [D, st*128] -> SBUF free layout (c, n_local)
                hcT_sb = hcT_pool.tile([P, KC * cfg.super_tile * P], bf16,
                                       tag="hcT")
                nc.sync.dma_start(
                    out=hcT_sb[:, : KC * st * P].rearrange(
                        "p (c n) -> p c n", c=KC),
                    in_=hcT[:, t0 * P:(t0 + st) * P].rearrange(
                        "(c p) n -> p c n", p=P),
                )
                for j in range(st):
                    t = t0 + j
                    hpl_sb = hpl_pool.tile([P, D], bf16, tag="hpl")
                    nc.sync.dma_start(out=hpl_sb[:],
                                      in_=hpl[t * P:(t + 1) * P, :])
                    # Gather the S sampled rows per node (HW indirect DMA
                    # honors one index per partition, so one call per s):
                    # g_sb[p, s*D:(s+1)*D] = hc[idx_sb[p, t*S+s], :]
                    g_sb = g_pool.tile([P, S * D], bf16, tag="g")
                    for s in range(S):
                        gi = nc.gpsimd.indirect_dma_start(
                            out=g_sb[:, s * D:(s + 1) * D],
                            out_offset=None,
                            in_=hc[:],
                            in_offset=bass.IndirectOffsetOnAxis(
                                ap=idx_sb[:, t * S + s:t * S + s + 1], axis=0),
                        )
                        # spread calls across SWDGE queues so SDMA
                        # interleaves descriptor streams
                        q = (t * S + s) % cfg.n_queues
                        if q:
                            gi.ins.queue = f"qPoolDynamic{q}"
                    # v = hc0_tile @ W.T via 4 accumulating bf16 matmuls
                    v_ps = psum_pool.tile([P, D], f32, space="PSUM", tag="v_ps")
                    for c in range(KC):
                        off = (c * st + j) * P
                        nc.tensor.matmul(
                            out=v_ps[:],
                            lhsT=hcT_sb[:, off:off + P],
                            rhs=wt_sb[:, c * D:(c + 1) * D],
                            start=(c == 0),
                            stop=(c == KC - 1),
                        )
                    col = t * (S + 1)
                    # sc1 and g0: fused multiply+reduce on DVE (one DVE op,
                    # no ACT involvement; InstTensorScalarPtr w/ accum_out)
                    for s, in0 in ((0, hpl_sb[:]), (1, g_sb[:, 0:D])):
                        stt_out = prod_pool.tile([P, D], bf16, tag="prod")
                        nc.vector.scalar_tensor_tensor(
                            out=stt_out[:],
                            in0=in0,
                            scalar=1.0,
                            in1=v_ps[:],
                            op0=mybir.AluOpType.mult,
                            op1=mybir.AluOpType.mult,
                            accum_out=sc_acc[:, col + s:col + s + 1],
                        )
                    # g1..g3: bf16 2x-mode multiply on DVE, reduce on ACT via
                    # Copy-activation accum_out
                    for s in range(2, S + 1):
                        prod = prod_pool.tile([P, D], bf16, tag="prod")
                        nc.vector.tensor_mul(prod[:], g_sb[:, (s - 1) * D:s * D],
                                             v_ps[:])
                        nc.scalar.activation(
                            dump[:], prod[:],
                            mybir.ActivationFunctionType.Copy,
                            accum_out=sc_acc[:, col + s:col + s + 1],
                        )
            nc.sync.dma_start(out=out[:], in_=sc_acc[:])
    nc.compile()
    return nc


def make_in_maps(cfg: Cfg, h_c, h_pl, sample_list, W, b):
    """Host-side sharding: full inputs -> per-core input dicts."""
    D, S = cfg.d, cfg.s
    hc0 = np.ascontiguousarray(np.asarray(h_c, np.float32)[0])
    hpl0 = np.asarray(h_pl, np.float32)[0]
    smp = np.asarray(sample_list)
    W0 = np.asarray(W, np.float32)[0]
    bval = float(np.asarray(b, np.float32).reshape(-1)[0])

    hc_bf = hc0.astype(ml_dtypes.bfloat16)             # gather table
    hcT = np.ascontiguousarray(hc0.T).astype(ml_dtypes.bfloat16)   # [D, N]
    wt = np.ascontiguousarray(W0.T).astype(ml_dtypes.bfloat16)

    in_maps = []
    for c in range(cfg.n_cores):
        lo = c * cfg.nodes_per_core
        hi = lo + cfg.nodes_per_core
        hcT_s = np.zeros((D, cfg.npad), ml_dtypes.bfloat16)
        hcT_s[:, : cfg.nodes_per_core] = hcT[:, lo:hi]
        hpl_s = np.zeros((cfg.npad, D), ml_dtypes.bfloat16)
        hpl_s[: cfg.nodes_per_core] = hpl0[lo:hi].astype(ml_dtypes.bfloat16)
        idx_s = np.zeros((S, cfg.npad), np.int64)
        idx_s[:, : cfg.nodes_per_core] = smp[:, lo:hi]
        idx_r = np.ascontiguousarray(
            idx_s.reshape(S, cfg.tiles, P).transpose(2, 1, 0)
            .astype(np.int32).reshape(P, cfg.tiles * S))
        in_maps.append({
            "hc": hc_bf, "hcT": hcT_s, "hpl": hpl_s,
            "idx": idx_r, "wt": wt,
        })
    return in_maps, bval


def assemble_output(cfg: Cfg, outs, bval):
    """Per-core 'out' arrays [P, TILES*(S+1)] -> full logits [1, N + 2*S*N].

    The +b bias is applied here (host-side) instead of on-device."""
    S = cfg.s
    n = cfg.nodes_per_core * cfg.n_cores
    sc1 = np.empty((n,), np.float32)
    sc2 = np.empty((S, n), np.float32)
    for c in range(cfg.n_cores):
        o = (outs[c].reshape(P, cfg.tiles, S + 1).transpose(2, 1, 0)
             .reshape(S + 1, cfg.npad)[:, : cfg.nodes_per_core])
        lo = c * cfg.nodes_per_core
        sc1[lo:lo + cfg.nodes_per_core] = o[0]
        sc2[:, lo:lo + cfg.nodes_per_core] = o[1:]
    flat = sc2.reshape(-1)
    res = np.concatenate([sc1, flat, flat])[None, :]
    if bval != 0.0:
        res = res + bval
    return res.astype(np.float32)


_NC_CACHE = {}


def _get_nc(cfg: Cfg):
    key = (cfg.n_table, cfg.nodes_per_core, cfg.d, cfg.s, cfg.super_tile,
           cfg.n_queues)
    if key not in _NC_CACHE:
        _NC_CACHE[key] = build_nc(cfg)
    return _NC_CACHE[key]


def run_on_hw(cfg: Cfg, inputs, trace=False, trace_kwargs={}):
    nc = _get_nc(cfg)
    in_maps, bval = make_in_maps(cfg, **inputs)
    res = run_bass_kernel_spmd(nc, in_maps, core_ids=list(range(cfg.n_cores)),
                               trace=trace, trace_kwargs=trace_kwargs)
    out = assemble_output(cfg, [r["out"] for r in res.results], bval)
    return out, res


def kernel(h_c, h_pl, sample_list, W, b):
    inputs = dict(h_c=h_c, h_pl=h_pl, sample_list=sample_list, W=W, b=b)
    out, _ = run_on_hw(FULL, inputs, trace=False)
    return out


if __name__ == "__main__":
    import reference

    inputs = reference.setup_inputs()
    expected = np.asarray(reference.reference(**inputs))
    got = kernel(**{k: np.asarray(v) for k, v in inputs.items()})
    rel = np.abs(got - expected).max() / np.abs(expected).max()
    print("Relative error:", rel)
